# revision 10
# baseline (speedup 1.0000x reference)
"""AttentionSAGEConv on 8 Trainium2 NeuronCores (Bass/Tile).

v3 design.  The hard floor on this stack is the per-edge K|V gather:
indirect DMA supports exactly 128 offsets ([128, 1]) per instruction
(multi-column offset APs silently mis-execute on HW), and each such
instruction costs ~1.1us of serial Pool-engine SWDGE time.  With
100k edges/core -> ~830 gather instructions -> ~920us of Pool time.
v3 therefore keeps the dense 128-edge block layout and optimizes
everything AROUND the gather train so it never stalls:

  - Phase 1 computes Q|K|V|x@Wm1 in one 512-wide matmul per 128-node
    tile; Q and x@Wm1 stay SBUF-resident (no DRAM round trip).
  - One AllGather materializes the fp16 [50000, 256] K|V table.
  - Phase 2 per 128-dst-node group: per-block one-hot machinery
    (Q expansion via PE transpose+matmul, segment-sums as accumulating
    fp16 matmuls into PSUM) with fp16 DVE datapaths, fp16 bias used
    directly, and the fused output
    out = relu(x @ Wm1 + agg_n @ (Wo @ Wm2) + (bo @ Wm2 + bm)).
  - Gather padding slots carry an out-of-bounds src id and are skipped
    via bounds_check (no HBM bytes); their one-hot columns are zero so
    they contribute exactly 0 (kvg buffers are memset once so no NaNs).
  - Deep buffering (kvg bufs=4, oh/wva bufs=3, PSUM pools bufs>=2) so
    the Pool engine's gather train runs back-to-back.
"""

import threading
import time
import numpy as np

N = 50000
E = 800000
IN_DIM = 128
OUT_DIM = 128
EDGE_DIM = 3
H = 4
HD = 32
SCALE = HD ** -0.5
NCORES = 8
NPC = N // NCORES          # nodes per core = 6250
G = (NPC + 127) // 128     # groups per core = 49
NPAD = G * 128             # padded nodes per core = 6272
PAD_SRC = 60000            # out-of-bounds src id for padding slots

_TLOG_ON = False


def _lap(msg, _t0=[None]):
    if not _TLOG_ON:
        return
    now = time.time()
    if _t0[0] is None:
        _t0[0] = now
    print(f"[k3] {msg} @ {now - _t0[0]:.2f}s", flush=True)


# ---- early terminal warm-up, started at import time ----
_WARM = {"dev": None, "err": None}
_WARM_EVT = threading.Event()


def _warmup_thread():
    try:
        import jax
        devs = jax.devices()
        z = jax.device_put(np.zeros((8,), np.float32), devs[0])
        z.block_until_ready()
        _WARM["dev"] = devs
    except Exception as e:  # pragma: no cover
        _WARM["err"] = e
    finally:
        _WARM_EVT.set()


_warm_th = threading.Thread(target=_warmup_thread, daemon=True)
_warm_th.start()


def _isa_thread():
    try:
        import concourse.isa as cisa
        cisa.get_isa("TRN2")
    except Exception:
        pass


_isa_th = threading.Thread(target=_isa_thread, daemon=True)
_isa_th.start()


def _patch_tile(tile_mod, mybir, ScopedClock):
    """This walrus build allows at most ONE semaphore wait per
    instruction.  Tile's final drain aggregates many waits; replace it
    with a chain of single-wait nops, and post-split every multi-wait
    instruction the Rust scheduler produced."""
    if getattr(tile_mod.TileContext, "_ant_drain_patched", False):
        return

    def _drain_and_barrier(self, tick_clock, wait_clock):
        probe = self.nc.sync.nop(nofuse=True)
        wait_clock.add_sem_waits(probe.ins, ScopedClock({None: tick_clock.global_clock}))
        si = probe.ins.sync_info
        waits = list(si.on_wait) if si is not None and si.on_wait else []
        if len(waits) > 1:
            probe.ins.sync_info = mybir.SyncInfo(on_wait=[waits[0]], on_update=[])
            for w in waits[1:]:
                n = self.nc.sync.nop(nofuse=True)
                n.ins.sync_info = mybir.SyncInfo(on_wait=[w], on_update=[])
        self.nc.sync.drain()
        self.nc.all_engine_barrier()
        popped = self.nc._tile_sem_poison_stack.pop()
        assert popped is self._sem_poison
        self.nc.clear_and_free_semaphores(list(self.sems.allocated().values()))
        self.nc.all_engine_barrier()

    tile_mod.TileContext._drain_and_barrier = _drain_and_barrier
    tile_mod.TileContext._ant_drain_patched = True


def _split_multi_waits(nc, mybir):
    for f in nc.m.functions:
        for blk in f.blocks:
            new = []
            for inst in blk.instructions:
                si = inst.sync_info
                if si is not None and si.on_wait and len(si.on_wait) > 1:
                    waits = list(si.on_wait)
                    for k, w in enumerate(waits[:-1]):
                        new.append(mybir.InstNoOp(
                            name=f"{inst.name}-ws{k}", engine=inst.engine,
                            sync_info=mybir.SyncInfo(on_wait=[w], on_update=[]),
                            bass_nofuse=True))
                    inst.sync_info = mybir.SyncInfo(
                        on_wait=[waits[-1]], on_update=list(si.on_update or []))
                new.append(inst)
            blk.instructions = new


def _prep(edge_index, edge_attr, We):
    """Host-side index prep.  Absolute src ids; per-core dst sort into
    128-node groups with one shared block structure; edge bias
    precomputed on host in fp16.  Padding slots: src id PAD_SRC (the
    gather's bounds check skips them -> no HBM traffic) and ldst -1
    (their one-hot column is zero -> exact zero contribution)."""
    src = np.asarray(edge_index[0], dtype=np.int64)
    dst = np.asarray(edge_index[1], dtype=np.int64)
    bias = (np.asarray(edge_attr, np.float32)
            @ np.asarray(We, np.float32)).astype(np.float16)  # [E, H]
    core = dst // NPC
    per_core = []
    counts_all = np.zeros((NCORES, G), dtype=np.int64)
    for c in range(NCORES):
        sel = np.nonzero(core == c)[0]
        d_loc = dst[sel] - c * NPC
        order = np.argsort(d_loc, kind="stable")
        sel = sel[order]
        d_loc = d_loc[order]
        counts = np.bincount(d_loc // 128, minlength=G)
        counts_all[c] = counts
        per_core.append((sel, d_loc, counts))

    nbs = ((counts_all.max(axis=0) + 127) // 128).astype(int)
    nbs = np.maximum(nbs, 1)
    b0s = np.concatenate([[0], np.cumsum(nbs)]).astype(int)
    B = int(b0s[-1])
    ins = []
    for c in range(NCORES):
        sel, d_loc, counts = per_core[c]
        srcidx = np.full((128, B), PAD_SRC, dtype=np.uint16)
        ldst = np.full((128, B), -1, dtype=np.int8)
        bia = np.zeros((128, B, H), dtype=np.float16)
        starts = np.concatenate([[0], np.cumsum(counts)])
        for g in range(G):
            e0, e1 = starts[g], starts[g + 1]
            idxs = sel[e0:e1]
            k = e1 - e0
            slot = np.arange(k)
            b = b0s[g] + slot // 128
            p = slot % 128
            srcidx[p, b] = src[idxs].astype(np.uint16)
            ldst[p, b] = (d_loc[e0:e1] - g * 128).astype(np.int8)
            bia[p, b, :] = bias[idxs]
        ins.append(dict(srcidx=srcidx, ldst=ldst, bias16=bia))
    return ins, nbs, b0s, B


def _build(nbs, b0s, B):
    import concourse.bass as bass
    import concourse.mybir as mybir
    import concourse.tile as tile
    from concourse.vector_clock import ScopedClock
    from concourse.masks import make_identity

    _patch_tile(tile, mybir, ScopedClock)
    f32 = mybir.dt.float32
    f16 = mybir.dt.float16
    AL = mybir.AluOpType
    AF = mybir.ActivationFunctionType

    nc = bass.Bass(target_bir_lowering=False, num_swdge_queues=4, num_devices=NCORES)
    # ---- per-core inputs ----
    xTl = nc.dram_tensor("xTl", [128, NPAD], f16, kind="ExternalInput")
    Wqkv = nc.dram_tensor("Wqkv", [128, 512], f16, kind="ExternalInput")
    W2 = nc.dram_tensor("W2", [128, 128], f16, kind="ExternalInput")
    b2r = nc.dram_tensor("b2r", [1, 128], f32, kind="ExternalInput")
    iota = nc.dram_tensor("iota", [128, 128], f16, kind="ExternalInput")
    srcidx = nc.dram_tensor("srcidx", [128, B], mybir.dt.uint16, kind="ExternalInput")
    ldst = nc.dram_tensor("ldst", [128, B], mybir.dt.int8, kind="ExternalInput")
    bias16 = nc.dram_tensor("bias16", [128, B, H], f16, kind="ExternalInput")
    out = nc.dram_tensor("out", [NPC, 128], f16, kind="ExternalOutput")
    # internal tables
    kvloc = nc.dram_tensor("kvloc", [NPC, 256], f16)
    kvt = nc.dram_tensor("kvt", [N, 256], f16, addr_space="Shared")

    NBMAX = int(max(nbs))

    with tile.TileContext(nc) as tc:
        with tc.tile_pool(name="const", bufs=1) as cpool, \
             tc.tile_pool(name="sb", bufs=3) as sb, \
             tc.tile_pool(name="sb2", bufs=3) as sb2, \
             tc.tile_pool(name="kvp", bufs=4) as kvp, \
             tc.tile_pool(name="ps", bufs=2, space="PSUM") as ps, \
             tc.tile_pool(name="psb", bufs=1, space="PSUM") as psb, \
             tc.tile_pool(name="pso", bufs=2, space="PSUM") as pso, \
             tc.tile_pool(name="ps1", bufs=2, space="PSUM") as ps1:

            # ---------- constants / setup ----------
            idt16 = cpool.tile([128, 128], f16)
            make_identity(nc, idt16[:])
            iota_sb = cpool.tile([128, 128], f16)
            nc.sync.dma_start(out=iota_sb[:], in_=iota[:])
            wqkv_sb = cpool.tile([128, 512], f16)
            nc.sync.dma_start(out=wqkv_sb[:], in_=Wqkv[:])
            w2_sb = cpool.tile([128, 128], f16)
            nc.sync.dma_start(out=w2_sb[:], in_=W2[:])
            b2_sb = cpool.tile([1, 128], f32)
            nc.sync.dma_start(out=b2_sb[:], in_=b2r[:])
            ones1 = cpool.tile([1, 128], f32)
            nc.vector.memset(ones1[:], 1.0)
            qsb = cpool.tile([128, G, 128], f16)
            xm1sb = cpool.tile([128, G, 128], f16)

            bias_sb = cpool.tile([128, B, H], f16)
            nc.sync.dma_start(out=bias_sb[:], in_=bias16[:])
            ldst8_sb = cpool.tile([128, B], mybir.dt.int8)
            nc.sync.dma_start(out=ldst8_sb[:], in_=ldst[:])
            ldst_sb = cpool.tile([128, B], f16)
            nc.vector.tensor_copy(out=ldst_sb[:], in_=ldst8_sb[:])
            srcidx16_sb = cpool.tile([128, B], mybir.dt.uint16)
            nc.sync.dma_start(out=srcidx16_sb[:], in_=srcidx[:])
            srcidx_sb = cpool.tile([128, B], mybir.dt.int32)
            nc.vector.tensor_copy(out=srcidx_sb[:], in_=srcidx16_sb[:])

            # zero the rotating K|V gather buffers once: padding slots
            # are never written (bounds-check skip), and stale *finite*
            # data is multiplied by a zero one-hot column, but
            # uninitialized SBUF could hold fp16 NaNs (0*NaN = NaN).
            for _ in range(4):
                kvz = kvp.tile([128, NBMAX, 256], f16, tag="kvg")
                nc.vector.memset(kvz[:], 0.0)

            # ---------- phase 1: Q|K|V|xWm1 for local nodes ----------
            chunk = 1024
            NCH = (NPAD + chunk - 1) // chunk  # 7 (last chunk = 128 cols)
            for t in range(NCH):
                r0 = t * chunk
                crows = min(chunk, NPAD - r0)
                nt = (crows + 127) // 128
                xt_t = sb.tile([128, chunk], f16, tag="p1x")
                nc.sync.dma_start(out=xt_t[:, :crows], in_=xTl[:, r0:r0 + crows])
                kvst = sb.tile([128, chunk // 128, 256], f16, tag="p1kv")
                for j in range(nt):
                    g = t * (chunk // 128) + j
                    pq = ps1.tile([128, 512], f32, tag="p1p")
                    nc.tensor.matmul(out=pq[:],
                                     lhsT=xt_t[:, j * 128:(j + 1) * 128],
                                     rhs=wqkv_sb[:], start=True, stop=True)
                    nc.scalar.copy(out=qsb[:, g, :], in_=pq[:, 0:128])
                    nc.vector.tensor_copy(out=kvst[:, j, :], in_=pq[:, 128:384])
                    nc.scalar.copy(out=xm1sb[:, g, :], in_=pq[:, 384:512])
                krows = min(crows, NPC - r0) if r0 < NPC else 0
                nfull = krows // 128
                if nfull:
                    nc.sync.dma_start(
                        out=kvloc[r0:r0 + nfull * 128, :]
                            .rearrange("(j p) f -> p j f", p=128),
                        in_=kvst[:, :nfull, :])
                if krows % 128:
                    jj = nfull
                    rows = krows % 128
                    nc.sync.dma_start(
                        out=kvloc[r0 + jj * 128:r0 + jj * 128 + rows, :],
                        in_=kvst[:rows, jj, :])

            # ---------- AllGather K|V across all 8 cores ----------
            nc.gpsimd.collective_compute(
                "AllGather", mybir.AluOpType.bypass,
                replica_groups=[list(range(NCORES))],
                ins=[kvloc[:].opt()], outs=[kvt[:].opt()])

            # ---------- phase 2 ----------
            bounds_reg = nc.gpsimd.to_reg(N - 1)
            for g in range(G):
                NB = int(nbs[g])
                b0 = int(b0s[g])
                rows = min(128, NPC - g * 128)

                kvg = kvp.tile([128, NBMAX, 256], f16, tag="kvg")
                for b in range(NB):
                    gi = nc.gpsimd.indirect_dma_start(
                        out=kvg[:, b, :], out_offset=None, in_=kvt[:],
                        in_offset=bass.IndirectOffsetOnAxis(
                            ap=srcidx_sb[:, b0 + b:b0 + b + 1], axis=0),
                        bounds_check=bounds_reg, oob_is_err=False)
                    qn = (b0 + b) % 4
                    if qn:
                        gi.ins.queue = f"qPoolDynamic{qn}"

                oh = sb2.tile([128, NBMAX, 128], f16, tag="oh")
                nc.vector.tensor_tensor(
                    out=oh[:, :NB, :],
                    in0=ldst_sb[:, b0:b0 + NB, None].to_broadcast([128, NB, 128]),
                    in1=iota_sb[:, None, :].to_broadcast([128, NB, 128]),
                    op=AL.is_equal)
                pk = sb2.tile([128, NBMAX, 128], f16, tag="pk")
                for b4 in range(0, NB, 4):
                    nb4 = min(4, NB - b4)
                    pqe = ps1.tile([128, 4, 128], f32, tag="p1p")
                    for j in range(nb4):
                        b = b4 + j
                        ptne = psb.tile([128, 128], f16, tag="ptne")
                        nc.tensor.transpose(out=ptne[:], in_=oh[:, b, :],
                                            identity=idt16[:])
                        ohT = sb.tile([128, 128], f16, tag="ohT")
                        nc.scalar.copy(out=ohT[:], in_=ptne[:])
                        nc.tensor.matmul(out=pqe[:, j, :], lhsT=ohT[:],
                                         rhs=qsb[:, g, :], start=True, stop=True)
                    nc.vector.tensor_tensor(out=pk[:, b4:b4 + nb4, :],
                                            in0=pqe[:, :nb4, :],
                                            in1=kvg[:, b4:b4 + nb4, 0:128],
                                            op=AL.mult)
                attnf = sb2.tile([128, NBMAX, H], f32, tag="attnf")
                # wv | attn_exp share one tile so the scatter needs a
                # single accumulating matmul per block
                wva = sb2.tile([128, NBMAX, 132], f16, tag="wva")
                nc.vector.tensor_reduce(
                    out=attnf[:, :NB, :],
                    in_=pk[:, :NB, :].rearrange("p b (h d) -> p (b h) d", d=HD),
                    axis=mybir.AxisListType.X, op=AL.add)
                nc.vector.scalar_tensor_tensor(
                    out=attnf[:, :NB, :], in0=attnf[:, :NB, :], scalar=SCALE,
                    in1=bias_sb[:, b0:b0 + NB, :], op0=AL.mult, op1=AL.add)
                nc.vector.scalar_tensor_tensor(
                    out=attnf[:, :NB, :], in0=attnf[:, :NB, :], scalar=0.2,
                    in1=attnf[:, :NB, :], op0=AL.mult, op1=AL.max)
                nc.scalar.activation(out=wva[:, :NB, 128:132],
                                     in_=attnf[:, :NB, :], func=AF.Exp)
                nc.vector.tensor_tensor(
                    out=wva[:, :NB, 0:128].rearrange("p b (h d) -> p b h d", d=HD),
                    in0=kvg[:, :NB, 128:256].rearrange("p b (h d) -> p b h d", d=HD),
                    in1=wva[:, :NB, 128:132, None].to_broadcast([128, NB, H, HD]),
                    op=AL.mult)

                pagg = ps.tile([128, 132], f32, tag="pagg")
                for b in range(NB):
                    nc.tensor.matmul(out=pagg[:], lhsT=oh[:, b, :], rhs=wva[:, b, :],
                                     start=(b == 0), stop=(b == NB - 1))

                sums = sb.tile([128, H], f32, tag="sums")
                nc.vector.tensor_scalar(out=sums[:], in0=pagg[:, 128:132],
                                        scalar1=1e-12, scalar2=None, op0=AL.max)
                rec = sb.tile([128, H], f32, tag="rec")
                nc.vector.reciprocal(out=rec[:], in_=sums[:])
                aggn = sb.tile([128, 128], f16, tag="aggn")
                nc.vector.tensor_tensor(
                    out=aggn[:].rearrange("p (h d) -> p h d", d=HD),
                    in0=pagg[:, 0:128].rearrange("p (h d) -> p h d", d=HD),
                    in1=rec[:, :, None].to_broadcast([128, H, HD]), op=AL.mult)

                ptr = psb.tile([128, 128], f16, tag="ptr")
                nc.tensor.transpose(out=ptr[:], in_=aggn[:], identity=idt16[:])
                aggnT = sb.tile([128, 128], f16, tag="aggnT")
                nc.scalar.copy(out=aggnT[:], in_=ptr[:])

                po = pso.tile([128, 128], f32, tag="po")
                nc.tensor.matmul(out=po[:], lhsT=aggnT[:], rhs=w2_sb[:],
                                 start=True, stop=False)
                nc.tensor.matmul(out=po[:], lhsT=idt16[:], rhs=xm1sb[:, g, :],
                                 start=False, stop=False)
                nc.tensor.matmul(out=po[:], lhsT=ones1[:], rhs=b2_sb[:],
                                 start=False, stop=True)
                osb = sb.tile([128, 128], f16, tag="osb")
                nc.scalar.activation(out=osb[:], in_=po[:], func=AF.Relu)
                nc.sync.dma_start(out=out[g * 128:g * 128 + rows, :],
                                  in_=osb[:rows, :])

    _split_multi_waits(nc, mybir)
    return nc


def _run_spmd_fast(nc, in_maps, n_cores):
    import jax
    from jax.sharding import Mesh, PartitionSpec, NamedSharding
    from jax.experimental.shard_map import shard_map
    import concourse.bass2jax as b2j
    import concourse.mybir as mybir

    b2j.install_neuronx_cc_hook()

    partition_name = nc.partition_id_tensor.name if nc.partition_id_tensor else None

    in_names, out_names, out_avals = [], [], []
    for alloc in nc.m.functions[0].allocations:
        if not isinstance(alloc, mybir.MemoryLocationSet):
            continue
        name = alloc.memorylocations[0].name
        if alloc.kind == "ExternalInput":
            if name != partition_name:
                in_names.append(name)
        elif alloc.kind == "ExternalOutput":
            out_names.append(name)
            shape = tuple(alloc.tensor_shape)
            dtype = mybir.dt.np(alloc.dtype)
            out_avals.append(jax.core.ShapedArray(shape, dtype))
    n_params = len(in_names)
    all_in_names = list(in_names)
    if partition_name is not None:
        all_in_names.append(partition_name)

    def _body(*args):
        operands = list(args)
        if partition_name is not None:
            operands.append(b2j.partition_id_tensor())
        outs = b2j._bass_exec_p.bind(
            *operands,
            out_avals=tuple(out_avals),
            in_names=tuple(all_in_names),
            out_names=tuple(out_names),
            lowering_input_output_aliases=(),
            sim_require_finite=True,
            sim_require_nnan=True,
            nc=nc,
        )
        return tuple(outs)

    devices = jax.devices()[:n_cores]
    mesh = Mesh(np.asarray(devices), ("core",))
    csh = NamedSharding(mesh, PartitionSpec("core"))
    in_specs = (PartitionSpec("core"),) * n_params
    out_specs = (PartitionSpec("core"),) * len(out_names)
    sharded = jax.jit(
        shard_map(_body, mesh=mesh, in_specs=in_specs, out_specs=out_specs,
                  check_rep=False),
        keep_unused=True,
    )

    # concat per-core inputs on host (cheap: small shards)
    concat_in = [
        np.concatenate([np.asarray(in_maps[c][nm]) for c in range(n_cores)], axis=0)
        for nm in in_names
    ]
    _lap("host concat done")

    # ---- background thread: wait for warm-up, then stream inputs ----
    dev_arrays = [None] * n_params
    thr_err = []

    def _stream():
        try:
            _WARM_EVT.wait()
            if _WARM["err"] is not None:
                # import-time warm-up failed (transient?) — retry inline
                z = jax.device_put(np.zeros((8,), np.float32), devices[0])
                z.block_until_ready()
                _WARM["err"] = None
            _lap("warmup ready")
            t0 = time.time()
            nb = 0
            for i, a in enumerate(concat_in):
                nb += a.nbytes
                dev_arrays[i] = jax.device_put(a, csh)
            for a in dev_arrays:
                a.block_until_ready()
            _lap(f"transfers done ({time.time()-t0:.2f}s, {nb/1e6:.1f}MB)")
        except Exception as e:  # pragma: no cover
            thr_err.append(e)

    th = threading.Thread(target=_stream, daemon=True)
    th.start()

    shapes = [jax.ShapeDtypeStruct(a.shape, a.dtype) for a in concat_in]
    t0 = time.time()
    compiled = sharded.lower(*shapes).compile()
    _lap(f"main lower+compile ({time.time()-t0:.2f}s)")

    th.join()
    if thr_err:
        raise thr_err[0]

    t0 = time.time()
    out_arrs = compiled(*dev_arrays)
    for o in out_arrs:
        o.block_until_ready()
    _lap(f"exec ({time.time()-t0:.2f}s)")
    t0 = time.time()
    host = [np.asarray(a).reshape(n_cores, *av.shape)
            for a, av in zip(out_arrs, out_avals)]
    res = [
        {name: host[i][c] for i, name in enumerate(out_names)}
        for c in range(n_cores)
    ]
    _lap(f"fetch ({time.time()-t0:.2f}s)")
    return res


def kernel(x, edge_index, edge_attr, Wq, Wk, Wv, We, Wo, bo, Wm, bm):
    _lap("kernel start")
    x = np.asarray(x, dtype=np.float32)
    prep_out = {}

    def _prep_job():
        try:
            prep_out["r"] = _prep(np.asarray(edge_index),
                                  np.asarray(edge_attr, np.float32), We)
        except Exception as e:
            prep_out["e"] = e

    pth = threading.Thread(target=_prep_job, daemon=True)
    pth.start()
    try:
        import concourse.isa as cisa
        cisa.get_isa("TRN2")
    except Exception:
        pass
    _lap("isa ready")
    pth.join()
    if "e" in prep_out:
        raise prep_out["e"]
    per_core, nbs, b0s, B = prep_out["r"]
    _lap("_prep done")

    nc = _build(nbs, b0s, B)
    _lap("_build done")

    Wm = np.asarray(Wm, np.float32)
    Wm2 = Wm[128:]
    W2 = (np.asarray(Wo, np.float32) @ Wm2).astype(np.float16)
    b2 = (np.asarray(bo, np.float32) @ Wm2 + np.asarray(bm, np.float32))
    Wqkv = np.concatenate(
        [np.asarray(Wq, np.float32), np.asarray(Wk, np.float32),
         np.asarray(Wv, np.float32), Wm[:128]], axis=1).astype(np.float16)
    common = dict(
        Wqkv=Wqkv,
        W2=W2,
        b2r=b2.reshape(1, 128).astype(np.float32),
        iota=np.tile(np.arange(128, dtype=np.float16)[None, :], (128, 1)),
    )
    xT16 = x.T.astype(np.float16)  # [128, N]
    in_maps = []
    for c in range(NCORES):
        m = dict(common)
        cols = np.zeros((128, NPAD), dtype=np.float16)
        cols[:, :NPC] = xT16[:, c * NPC:(c + 1) * NPC]
        m["xTl"] = cols
        m.update(per_core[c])
        in_maps.append(m)
    _lap("in_maps done")

    t0 = time.time()
    res = _run_spmd_fast(nc, in_maps, NCORES)
    global _LAST_RUN_NS, _LAST_NC, _LAST_IN_MAPS
    _LAST_RUN_NS = int((time.time() - t0) * 1e9)
    _LAST_NC = nc
    _LAST_IN_MAPS = in_maps
    outs = [res[c]["out"] for c in range(NCORES)]
    return np.concatenate(outs, axis=0).astype(np.float32)


_LAST_RUN_NS = None
_LAST_NC = None
_LAST_IN_MAPS = None


# revision 17
# speedup vs baseline: 1.2692x; 1.2692x over previous
"""AttentionSAGEConv on 8 Trainium2 NeuronCores (Bass/Tile).

v3 design.  The hard floor on this stack is the per-edge K|V gather:
indirect DMA supports exactly 128 offsets ([128, 1]) per instruction
(multi-column offset APs silently mis-execute on HW), and each such
instruction costs ~1.1us of serial Pool-engine SWDGE time.  With
100k edges/core -> ~830 gather instructions -> ~920us of Pool time.
v3 therefore keeps the dense 128-edge block layout and optimizes
everything AROUND the gather train so it never stalls:

  - Phase 1 computes Q|K|V|x@Wm1 in one 512-wide matmul per 128-node
    tile; Q and x@Wm1 stay SBUF-resident (no DRAM round trip).
  - One AllGather materializes the fp16 [50000, 256] K|V table.
  - Phase 2 per 128-dst-node group: per-block one-hot machinery
    (Q expansion via PE transpose+matmul, segment-sums as accumulating
    fp16 matmuls into PSUM) with fp16 DVE datapaths, fp16 bias used
    directly, and the fused output
    out = relu(x @ Wm1 + agg_n @ (Wo @ Wm2) + (bo @ Wm2 + bm)).
  - Gather padding slots carry an out-of-bounds src id and are skipped
    via bounds_check (no HBM bytes); their one-hot columns are zero so
    they contribute exactly 0 (kvg buffers are memset once so no NaNs).
  - Deep buffering (kvg bufs=4, oh/wva bufs=3, PSUM pools bufs>=2) so
    the Pool engine's gather train runs back-to-back.
"""

import threading
import time
import numpy as np

N = 50000
E = 800000
IN_DIM = 128
OUT_DIM = 128
EDGE_DIM = 3
H = 4
HD = 32
SCALE = HD ** -0.5
NCORES = 8
NPC = N // NCORES          # nodes per core = 6250
G = (NPC + 127) // 128     # groups per core = 49
NPAD = G * 128             # padded nodes per core = 6272
PAD_SRC = 60000            # out-of-bounds src id for padding slots

_TLOG_ON = False


def _lap(msg, _t0=[None]):
    if not _TLOG_ON:
        return
    now = time.time()
    if _t0[0] is None:
        _t0[0] = now
    print(f"[k3] {msg} @ {now - _t0[0]:.2f}s", flush=True)


# ---- early terminal warm-up, started at import time ----
_WARM = {"dev": None, "err": None}
_WARM_EVT = threading.Event()


def _warmup_thread():
    try:
        import jax
        devs = jax.devices()
        z = jax.device_put(np.zeros((8,), np.float32), devs[0])
        z.block_until_ready()
        _WARM["dev"] = devs
    except Exception as e:  # pragma: no cover
        _WARM["err"] = e
    finally:
        _WARM_EVT.set()


_warm_th = threading.Thread(target=_warmup_thread, daemon=True)
_warm_th.start()


def _isa_thread():
    try:
        import concourse.isa as cisa
        cisa.get_isa("TRN2")
    except Exception:
        pass


_isa_th = threading.Thread(target=_isa_thread, daemon=True)
_isa_th.start()


def _patch_tile(tile_mod, mybir, ScopedClock):
    """This walrus build allows at most ONE semaphore wait per
    instruction.  Tile's final drain aggregates many waits; replace it
    with a chain of single-wait nops, and post-split every multi-wait
    instruction the Rust scheduler produced."""
    if getattr(tile_mod.TileContext, "_ant_drain_patched", False):
        return

    def _drain_and_barrier(self, tick_clock, wait_clock):
        probe = self.nc.sync.nop(nofuse=True)
        wait_clock.add_sem_waits(probe.ins, ScopedClock({None: tick_clock.global_clock}))
        si = probe.ins.sync_info
        waits = list(si.on_wait) if si is not None and si.on_wait else []
        if len(waits) > 1:
            probe.ins.sync_info = mybir.SyncInfo(on_wait=[waits[0]], on_update=[])
            for w in waits[1:]:
                n = self.nc.sync.nop(nofuse=True)
                n.ins.sync_info = mybir.SyncInfo(on_wait=[w], on_update=[])
        self.nc.sync.drain()
        self.nc.all_engine_barrier()
        popped = self.nc._tile_sem_poison_stack.pop()
        assert popped is self._sem_poison
        self.nc.clear_and_free_semaphores(list(self.sems.allocated().values()))
        self.nc.all_engine_barrier()

    tile_mod.TileContext._drain_and_barrier = _drain_and_barrier
    tile_mod.TileContext._ant_drain_patched = True


def _split_multi_waits(nc, mybir):
    for f in nc.m.functions:
        for blk in f.blocks:
            new = []
            for inst in blk.instructions:
                si = inst.sync_info
                if si is not None and si.on_wait and len(si.on_wait) > 1:
                    waits = list(si.on_wait)
                    for k, w in enumerate(waits[:-1]):
                        new.append(mybir.InstNoOp(
                            name=f"{inst.name}-ws{k}", engine=inst.engine,
                            sync_info=mybir.SyncInfo(on_wait=[w], on_update=[]),
                            bass_nofuse=True))
                    inst.sync_info = mybir.SyncInfo(
                        on_wait=[waits[-1]], on_update=list(si.on_update or []))
                new.append(inst)
            blk.instructions = new


def _prep(edge_index, edge_attr, We):
    """Host-side index prep.  Absolute src ids; per-core dst sort into
    128-node groups with one shared block structure; edge bias
    precomputed on host in fp16.  Padding slots: src id PAD_SRC (the
    gather's bounds check skips them -> no HBM traffic) and ldst -1
    (their one-hot column is zero -> exact zero contribution)."""
    src = np.asarray(edge_index[0], dtype=np.int64)
    dst = np.asarray(edge_index[1], dtype=np.int64)
    bias = (np.asarray(edge_attr, np.float32)
            @ np.asarray(We, np.float32)).astype(np.float16)  # [E, H]
    core = dst // NPC
    per_core = []
    counts_all = np.zeros((NCORES, G), dtype=np.int64)
    nloc_all = np.zeros((NCORES, G), dtype=np.int64)
    for c in range(NCORES):
        sel = np.nonzero(core == c)[0]
        d_loc = dst[sel] - c * NPC
        # order edges within each group local-src-first, so a uniform
        # prefix of whole 128-edge columns can be gathered from the
        # local kvloc table BEFORE the AllGather completes
        is_rem = (src[sel] // NPC) != c
        order = np.lexsort((is_rem.astype(np.int8), d_loc // 128))
        sel = sel[order]
        d_loc = d_loc[order]
        counts = np.bincount(d_loc // 128, minlength=G)
        counts_all[c] = counts
        nloc = np.bincount((d_loc // 128)[~is_rem[order]], minlength=G)
        nloc_all[c] = nloc
        per_core.append((sel, d_loc, counts))

    nbs = ((counts_all.max(axis=0) + 127) // 128).astype(int)
    nbs = np.maximum(nbs, 1)
    nlocs = (nloc_all.min(axis=0) // 128).astype(int)  # pure-local cols
    b0s = np.concatenate([[0], np.cumsum(nbs)]).astype(int)
    B = int(b0s[-1])
    ins = []
    for c in range(NCORES):
        sel, d_loc, counts = per_core[c]
        srcidx = np.zeros((128, B), dtype=np.uint16)
        ldst = np.full((128, B), -1, dtype=np.int8)
        bia = np.zeros((128, B, H), dtype=np.float16)
        starts = np.concatenate([[0], np.cumsum(counts)])
        for g in range(G):
            e0, e1 = starts[g], starts[g + 1]
            idxs = sel[e0:e1]
            k = e1 - e0
            slot = np.arange(k)
            b = b0s[g] + slot // 128
            p = slot % 128
            sv = src[idxs].copy()
            # local-prefix columns are gathered from kvloc: local index
            lcols = nlocs[g]
            if lcols:
                inloc = (slot // 128) < lcols
                sv[inloc] -= c * NPC
            srcidx[p, b] = sv.astype(np.uint16)
            ldst[p, b] = (d_loc[e0:e1] - g * 128).astype(np.int8)
            bia[p, b, :] = bias[idxs]
        ins.append(dict(srcidx=srcidx, ldst=ldst, bias16=bia))
    return ins, nbs, nlocs, b0s, B


def _build(nbs, nlocs, b0s, B):
    import concourse.bass as bass
    import concourse.mybir as mybir
    import concourse.tile as tile
    from concourse.vector_clock import ScopedClock
    from concourse.masks import make_identity

    _patch_tile(tile, mybir, ScopedClock)
    f32 = mybir.dt.float32
    f16 = mybir.dt.float16
    AL = mybir.AluOpType
    AF = mybir.ActivationFunctionType
    l0s = np.concatenate([[0], np.cumsum(nlocs)]).astype(int)
    NLOCT = int(l0s[-1])

    nc = bass.Bass(target_bir_lowering=False, num_swdge_queues=4, num_devices=NCORES)
    # ---- per-core inputs ----
    xTl = nc.dram_tensor("xTl", [128, NPAD], f16, kind="ExternalInput")
    Wqkv = nc.dram_tensor("Wqkv", [128, 512], f16, kind="ExternalInput")
    W2 = nc.dram_tensor("W2", [128, 128], f16, kind="ExternalInput")
    b2r = nc.dram_tensor("b2r", [1, 128], f32, kind="ExternalInput")
    iota = nc.dram_tensor("iota", [128, 128], f16, kind="ExternalInput")
    srcidx = nc.dram_tensor("srcidx", [128, B], mybir.dt.uint16, kind="ExternalInput")
    ldst = nc.dram_tensor("ldst", [128, B], mybir.dt.int8, kind="ExternalInput")
    bias16 = nc.dram_tensor("bias16", [128, B, H], f16, kind="ExternalInput")
    out = nc.dram_tensor("out", [NPC, 128], f16, kind="ExternalOutput")
    # internal tables
    kvloc = nc.dram_tensor("kvloc", [NPC, 256], f16)
    kvt = nc.dram_tensor("kvt", [N, 256], f16, addr_space="Shared")

    NBMAX = int(max(nbs))

    with tile.TileContext(nc) as tc:
        with tc.tile_pool(name="const", bufs=1) as cpool, \
             tc.tile_pool(name="sb", bufs=3) as sb, \
             tc.tile_pool(name="sb2", bufs=3) as sb2, \
             tc.tile_pool(name="kvp", bufs=4) as kvp, \
             tc.tile_pool(name="ps", bufs=2, space="PSUM") as ps, \
             tc.tile_pool(name="psb", bufs=1, space="PSUM") as psb, \
             tc.tile_pool(name="pso", bufs=2, space="PSUM") as pso, \
             tc.tile_pool(name="ps1", bufs=2, space="PSUM") as ps1:

            # ---------- constants / setup ----------
            # src indices first: the local-column gathers need them
            # right after phase 1, during the AllGather
            srcidx16_sb = cpool.tile([128, B], mybir.dt.uint16)
            nc.sync.dma_start(out=srcidx16_sb[:], in_=srcidx[:])
            srcidx_sb = cpool.tile([128, B], mybir.dt.int32)
            nc.vector.tensor_copy(out=srcidx_sb[:], in_=srcidx16_sb[:])
            idt16 = cpool.tile([128, 128], f16)
            make_identity(nc, idt16[:])
            iota_sb = cpool.tile([128, 128], f16)
            nc.sync.dma_start(out=iota_sb[:], in_=iota[:])
            wqkv_sb = cpool.tile([128, 512], f16)
            nc.sync.dma_start(out=wqkv_sb[:], in_=Wqkv[:])
            w2_sb = cpool.tile([128, 128], f16)
            nc.sync.dma_start(out=w2_sb[:], in_=W2[:])
            b2_sb = cpool.tile([1, 128], f32)
            nc.sync.dma_start(out=b2_sb[:], in_=b2r[:])
            ones1 = cpool.tile([1, 128], f32)
            nc.vector.memset(ones1[:], 1.0)
            qsb = cpool.tile([128, G, 128], f16)
            xm1sb = cpool.tile([128, G, 128], f16)
            kvgl = cpool.tile([128, max(NLOCT, 1), 256], f16)
            bias_sb = cpool.tile([128, B, H], f16)
            ldst8_sb = cpool.tile([128, B], mybir.dt.int8)
            ldst_sb = cpool.tile([128, B], f16)

            # zero the rotating K|V gather buffers once: padding slots
            # are never written (bounds-check skip), and stale *finite*
            # data is multiplied by a zero one-hot column, but
            # uninitialized SBUF could hold fp16 NaNs (0*NaN = NaN).
            for _ in range(4):
                kvz = kvp.tile([128, NBMAX, 256], f16, tag="kvg")
                nc.vector.memset(kvz[:], 0.0)

            # ---------- phase 1: Q|K|V|xWm1 for local nodes ----------
            chunk = 1024
            NCH = (NPAD + chunk - 1) // chunk  # 7 (last chunk = 128 cols)
            for t in range(NCH):
                r0 = t * chunk
                crows = min(chunk, NPAD - r0)
                nt = (crows + 127) // 128
                xt_t = sb.tile([128, chunk], f16, tag="p1x")
                nc.sync.dma_start(out=xt_t[:, :crows], in_=xTl[:, r0:r0 + crows])
                kvst = sb.tile([128, chunk // 128, 256], f16, tag="p1kv")
                for j in range(nt):
                    g = t * (chunk // 128) + j
                    pq = ps1.tile([128, 512], f32, tag="p1p")
                    nc.tensor.matmul(out=pq[:],
                                     lhsT=xt_t[:, j * 128:(j + 1) * 128],
                                     rhs=wqkv_sb[:], start=True, stop=True)
                    nc.scalar.copy(out=qsb[:, g, :], in_=pq[:, 0:128])
                    nc.vector.tensor_copy(out=kvst[:, j, :], in_=pq[:, 128:384])
                    nc.scalar.copy(out=xm1sb[:, g, :], in_=pq[:, 384:512])
                krows = min(crows, NPC - r0) if r0 < NPC else 0
                nfull = krows // 128
                if nfull:
                    nc.sync.dma_start(
                        out=kvloc[r0:r0 + nfull * 128, :]
                            .rearrange("(j p) f -> p j f", p=128),
                        in_=kvst[:, :nfull, :])
                if krows % 128:
                    jj = nfull
                    rows = krows % 128
                    nc.sync.dma_start(
                        out=kvloc[r0 + jj * 128:r0 + jj * 128 + rows, :],
                        in_=kvst[:rows, jj, :])

            # ---------- AllGather K|V across all 8 cores ----------
            nc.gpsimd.collective_compute(
                "AllGather", mybir.AluOpType.bypass,
                replica_groups=[list(range(NCORES))],
                ins=[kvloc[:].opt()], outs=[kvt[:].opt()])

            # local-prefix columns: every edge in them has a local src,
            # so they gather from kvloc and run DURING the AllGather
            qrot = 0
            for g in range(G):
                b0 = int(b0s[g])
                for j in range(int(nlocs[g])):
                    gi = nc.gpsimd.indirect_dma_start(
                        out=kvgl[:, int(l0s[g]) + j, :], out_offset=None,
                        in_=kvloc[:],
                        in_offset=bass.IndirectOffsetOnAxis(
                            ap=srcidx_sb[:, b0 + j:b0 + j + 1], axis=0))
                    qrot += 1
                    if qrot % 4:
                        gi.ins.queue = f"qPoolDynamic{qrot % 4}"

            # remaining phase-2 inputs (only needed by compute, after
            # the collective; keeps them off the phase-1 DMA path)
            nc.sync.dma_start(out=bias_sb[:], in_=bias16[:])
            nc.sync.dma_start(out=ldst8_sb[:], in_=ldst[:])
            nc.vector.tensor_copy(out=ldst_sb[:], in_=ldst8_sb[:])

            # ---------- phase 2 ----------
            for g in range(G):
                NB = int(nbs[g])
                b0 = int(b0s[g])
                lc = int(nlocs[g])
                l0 = int(l0s[g])
                rows = min(128, NPC - g * 128)

                kvg = kvp.tile([128, NBMAX, 256], f16, tag="kvg")
                for b in range(lc, NB):
                    gi = nc.gpsimd.indirect_dma_start(
                        out=kvg[:, b - lc, :], out_offset=None, in_=kvt[:],
                        in_offset=bass.IndirectOffsetOnAxis(
                            ap=srcidx_sb[:, b0 + b:b0 + b + 1], axis=0))
                    qrot += 1
                    if qrot % 4:
                        gi.ins.queue = f"qPoolDynamic{qrot % 4}"

                runs = ([(0, lc)] if lc else []) + [(lc, NB)]

                oh = sb2.tile([128, NBMAX, 128], f16, tag="oh")
                nc.vector.tensor_tensor(
                    out=oh[:, :NB, :],
                    in0=ldst_sb[:, b0:b0 + NB, None].to_broadcast([128, NB, 128]),
                    in1=iota_sb[:, None, :].to_broadcast([128, NB, 128]),
                    op=AL.is_equal)
                pk = sb2.tile([128, NBMAX, 128], f16, tag="pk")
                for r0, r1 in runs:
                    for b4 in range(r0, r1, 4):
                        nb4 = min(4, r1 - b4)
                        pqe = ps1.tile([128, 4, 128], f32, tag="p1p")
                        for j in range(nb4):
                            b = b4 + j
                            ptne = psb.tile([128, 128], f16, tag="ptne")
                            nc.tensor.transpose(out=ptne[:], in_=oh[:, b, :],
                                                identity=idt16[:])
                            ohT = sb.tile([128, 128], f16, tag="ohT")
                            nc.scalar.copy(out=ohT[:], in_=ptne[:])
                            nc.tensor.matmul(out=pqe[:, j, :], lhsT=ohT[:],
                                             rhs=qsb[:, g, :], start=True, stop=True)
                        if b4 < lc:
                            ksrc = kvgl[:, l0 + b4:l0 + b4 + nb4, 0:128]
                        else:
                            ksrc = kvg[:, b4 - lc:b4 - lc + nb4, 0:128]
                        nc.vector.tensor_tensor(out=pk[:, b4:b4 + nb4, :],
                                                in0=pqe[:, :nb4, :],
                                                in1=ksrc, op=AL.mult)
                attnf = sb2.tile([128, NBMAX, H], f32, tag="attnf")
                # wv | attn_exp share one tile so the scatter needs a
                # single accumulating matmul per block
                wva = sb2.tile([128, NBMAX, 132], f16, tag="wva")
                nc.vector.tensor_reduce(
                    out=attnf[:, :NB, :],
                    in_=pk[:, :NB, :].rearrange("p b (h d) -> p (b h) d", d=HD),
                    axis=mybir.AxisListType.X, op=AL.add)
                nc.vector.scalar_tensor_tensor(
                    out=attnf[:, :NB, :], in0=attnf[:, :NB, :], scalar=SCALE,
                    in1=bias_sb[:, b0:b0 + NB, :], op0=AL.mult, op1=AL.add)
                nc.vector.scalar_tensor_tensor(
                    out=attnf[:, :NB, :], in0=attnf[:, :NB, :], scalar=0.2,
                    in1=attnf[:, :NB, :], op0=AL.mult, op1=AL.max)
                nc.scalar.activation(out=wva[:, :NB, 128:132],
                                     in_=attnf[:, :NB, :], func=AF.Exp)
                for r0, r1 in runs:
                    if r0 < lc:
                        vsrc = kvgl[:, l0 + r0:l0 + r1, 128:256]
                    else:
                        vsrc = kvg[:, r0 - lc:r1 - lc, 128:256]
                    nc.vector.tensor_tensor(
                        out=wva[:, r0:r1, 0:128]
                            .rearrange("p b (h d) -> p b h d", d=HD),
                        in0=vsrc.rearrange("p b (h d) -> p b h d", d=HD),
                        in1=wva[:, r0:r1, 128:132, None]
                            .to_broadcast([128, r1 - r0, H, HD]),
                        op=AL.mult)

                pagg = ps.tile([128, 132], f32, tag="pagg")
                for b in range(NB):
                    nc.tensor.matmul(out=pagg[:], lhsT=oh[:, b, :], rhs=wva[:, b, :],
                                     start=(b == 0), stop=(b == NB - 1))

                sums = sb.tile([128, H], f32, tag="sums")
                nc.vector.tensor_scalar(out=sums[:], in0=pagg[:, 128:132],
                                        scalar1=1e-12, scalar2=None, op0=AL.max)
                rec = sb.tile([128, H], f32, tag="rec")
                nc.vector.reciprocal(out=rec[:], in_=sums[:])
                aggn = sb.tile([128, 128], f16, tag="aggn")
                nc.vector.tensor_tensor(
                    out=aggn[:].rearrange("p (h d) -> p h d", d=HD),
                    in0=pagg[:, 0:128].rearrange("p (h d) -> p h d", d=HD),
                    in1=rec[:, :, None].to_broadcast([128, H, HD]), op=AL.mult)

                ptr = psb.tile([128, 128], f16, tag="ptr")
                nc.tensor.transpose(out=ptr[:], in_=aggn[:], identity=idt16[:])
                aggnT = sb.tile([128, 128], f16, tag="aggnT")
                nc.scalar.copy(out=aggnT[:], in_=ptr[:])

                po = pso.tile([128, 128], f32, tag="po")
                nc.tensor.matmul(out=po[:], lhsT=aggnT[:], rhs=w2_sb[:],
                                 start=True, stop=False)
                nc.tensor.matmul(out=po[:], lhsT=idt16[:], rhs=xm1sb[:, g, :],
                                 start=False, stop=False)
                nc.tensor.matmul(out=po[:], lhsT=ones1[:], rhs=b2_sb[:],
                                 start=False, stop=True)
                osb = sb.tile([128, 128], f16, tag="osb")
                nc.scalar.activation(out=osb[:], in_=po[:], func=AF.Relu)
                nc.sync.dma_start(out=out[g * 128:g * 128 + rows, :],
                                  in_=osb[:rows, :])

    _split_multi_waits(nc, mybir)
    return nc


def _run_spmd_fast(nc, in_maps, n_cores):
    import jax
    from jax.sharding import Mesh, PartitionSpec, NamedSharding
    from jax.experimental.shard_map import shard_map
    import concourse.bass2jax as b2j
    import concourse.mybir as mybir

    b2j.install_neuronx_cc_hook()

    partition_name = nc.partition_id_tensor.name if nc.partition_id_tensor else None

    in_names, out_names, out_avals = [], [], []
    for alloc in nc.m.functions[0].allocations:
        if not isinstance(alloc, mybir.MemoryLocationSet):
            continue
        name = alloc.memorylocations[0].name
        if alloc.kind == "ExternalInput":
            if name != partition_name:
                in_names.append(name)
        elif alloc.kind == "ExternalOutput":
            out_names.append(name)
            shape = tuple(alloc.tensor_shape)
            dtype = mybir.dt.np(alloc.dtype)
            out_avals.append(jax.core.ShapedArray(shape, dtype))
    n_params = len(in_names)
    all_in_names = list(in_names)
    if partition_name is not None:
        all_in_names.append(partition_name)

    def _body(*args):
        operands = list(args)
        if partition_name is not None:
            operands.append(b2j.partition_id_tensor())
        outs = b2j._bass_exec_p.bind(
            *operands,
            out_avals=tuple(out_avals),
            in_names=tuple(all_in_names),
            out_names=tuple(out_names),
            lowering_input_output_aliases=(),
            sim_require_finite=True,
            sim_require_nnan=True,
            nc=nc,
        )
        return tuple(outs)

    devices = jax.devices()[:n_cores]
    mesh = Mesh(np.asarray(devices), ("core",))
    csh = NamedSharding(mesh, PartitionSpec("core"))
    in_specs = (PartitionSpec("core"),) * n_params
    out_specs = (PartitionSpec("core"),) * len(out_names)
    sharded = jax.jit(
        shard_map(_body, mesh=mesh, in_specs=in_specs, out_specs=out_specs,
                  check_rep=False),
        keep_unused=True,
    )

    # concat per-core inputs on host (cheap: small shards)
    concat_in = [
        np.concatenate([np.asarray(in_maps[c][nm]) for c in range(n_cores)], axis=0)
        for nm in in_names
    ]
    _lap("host concat done")

    # ---- background thread: wait for warm-up, then stream inputs ----
    dev_arrays = [None] * n_params
    thr_err = []

    def _stream():
        try:
            _WARM_EVT.wait()
            if _WARM["err"] is not None:
                # import-time warm-up failed (transient?) — retry inline
                z = jax.device_put(np.zeros((8,), np.float32), devices[0])
                z.block_until_ready()
                _WARM["err"] = None
            _lap("warmup ready")
            t0 = time.time()
            nb = 0
            for i, a in enumerate(concat_in):
                nb += a.nbytes
                dev_arrays[i] = jax.device_put(a, csh)
            for a in dev_arrays:
                a.block_until_ready()
            _lap(f"transfers done ({time.time()-t0:.2f}s, {nb/1e6:.1f}MB)")
        except Exception as e:  # pragma: no cover
            thr_err.append(e)

    th = threading.Thread(target=_stream, daemon=True)
    th.start()

    shapes = [jax.ShapeDtypeStruct(a.shape, a.dtype) for a in concat_in]
    t0 = time.time()
    compiled = sharded.lower(*shapes).compile()
    _lap(f"main lower+compile ({time.time()-t0:.2f}s)")

    th.join()
    if thr_err:
        raise thr_err[0]

    t0 = time.time()
    out_arrs = compiled(*dev_arrays)
    for o in out_arrs:
        o.block_until_ready()
    _lap(f"exec ({time.time()-t0:.2f}s)")
    t0 = time.time()
    host = [np.asarray(a).reshape(n_cores, *av.shape)
            for a, av in zip(out_arrs, out_avals)]
    res = [
        {name: host[i][c] for i, name in enumerate(out_names)}
        for c in range(n_cores)
    ]
    _lap(f"fetch ({time.time()-t0:.2f}s)")
    return res


def kernel(x, edge_index, edge_attr, Wq, Wk, Wv, We, Wo, bo, Wm, bm):
    _lap("kernel start")
    x = np.asarray(x, dtype=np.float32)
    prep_out = {}

    def _prep_job():
        try:
            prep_out["r"] = _prep(np.asarray(edge_index),
                                  np.asarray(edge_attr, np.float32), We)
        except Exception as e:
            prep_out["e"] = e

    pth = threading.Thread(target=_prep_job, daemon=True)
    pth.start()
    try:
        import concourse.isa as cisa
        cisa.get_isa("TRN2")
    except Exception:
        pass
    _lap("isa ready")
    pth.join()
    if "e" in prep_out:
        raise prep_out["e"]
    per_core, nbs, nlocs, b0s, B = prep_out["r"]
    _lap("_prep done")

    nc = _build(nbs, nlocs, b0s, B)
    _lap("_build done")

    Wm = np.asarray(Wm, np.float32)
    Wm2 = Wm[128:]
    W2 = (np.asarray(Wo, np.float32) @ Wm2).astype(np.float16)
    b2 = (np.asarray(bo, np.float32) @ Wm2 + np.asarray(bm, np.float32))
    Wqkv = np.concatenate(
        [np.asarray(Wq, np.float32), np.asarray(Wk, np.float32),
         np.asarray(Wv, np.float32), Wm[:128]], axis=1).astype(np.float16)
    common = dict(
        Wqkv=Wqkv,
        W2=W2,
        b2r=b2.reshape(1, 128).astype(np.float32),
        iota=np.tile(np.arange(128, dtype=np.float16)[None, :], (128, 1)),
    )
    xT16 = x.T.astype(np.float16)  # [128, N]
    in_maps = []
    for c in range(NCORES):
        m = dict(common)
        cols = np.zeros((128, NPAD), dtype=np.float16)
        cols[:, :NPC] = xT16[:, c * NPC:(c + 1) * NPC]
        m["xTl"] = cols
        m.update(per_core[c])
        in_maps.append(m)
    _lap("in_maps done")

    t0 = time.time()
    res = _run_spmd_fast(nc, in_maps, NCORES)
    global _LAST_RUN_NS, _LAST_NC, _LAST_IN_MAPS
    _LAST_RUN_NS = int((time.time() - t0) * 1e9)
    _LAST_NC = nc
    _LAST_IN_MAPS = in_maps
    outs = [res[c]["out"] for c in range(NCORES)]
    return np.concatenate(outs, axis=0).astype(np.float32)


_LAST_RUN_NS = None
_LAST_NC = None
_LAST_IN_MAPS = None
